# revision 26
# baseline (speedup 1.0000x reference)
"""Embedding-lookup kernel for TRN2 (8 NeuronCores, batch-parallel).

Computation (per batch element b, K=6 targets, EMB=128):
    x[b]      = D[doc_ids[b]] + sum_c W[ctx_ids[b, c]]
    out[b, k] = x[b] . Wp[:, tn_ids[b, k]]

Sharding: data-parallel over batch (B=16384 -> 2048 per core); D, W and
Wp^T replicated on every core.

Device strategy per core (all gathers move 512B rows):
  Stage A (x):
    - D rows: 16x [P,1] indirect_dma_start (int32 ids) into a
      batch-aligned xD tile.
    - W ctx rows: dma_gather indices are int16, so the 100001-row table
      splits into 4 banks of 32768. Jobs are bank-sorted on the host.
      Per bank: dma_gather (compact list, trailing -1 pads, runtime
      count) -> staging -> plain DMA to a contiguous HBM scratch
      (position compression: scratch slot = bank-sorted position, all
      slots < 17408 fit int16).
    - ONE re-gather pass (4 chunks) pulls scratch rows in a
      striping-corrected (b, c) order so each batch element's 8 rows
      land in its own partition; DVE reduces over c and adds xD -> x.
    - x -> x_hbm scratch table (row p*16+m = batch m*128+p).
  Stage B (dots):
    - Wp^T jobs bank-sorted likewise. Per bank: dma_gather Wp^T rows ->
      Y staging; dma_gather x rows from x_hbm in the SAME job order -> X
      staging (aligned by construction; x row ids < 2048 fit int16).
    - DVE: X *= Y, reduce over emb -> dots in job order; host unpermutes.
"""

import sys

sys.path.insert(0, "/opt/trn_rl_repo")

from contextlib import ExitStack

import numpy as np

from concourse import bacc, bass, mybir
from concourse.bass_utils import run_bass_kernel_spmd
from concourse.library_config import mlp

N_CORES = 8
B = 16384
B_LOC = B // N_CORES  # 2048
P = 128
M = B_LOC // P  # 16 batch elements per partition
CTX = 8
K = 6
EMB = 128
N_DOCS = 500000
N_WORDS = 100000

BANK = 32768
W_BANKS = 4
WP_BANKS = 4
W_CAPS = [5888, 5888, 5888, 640]  # ctx jobs: 16384 total; bank3 holds 1697 rows
WP_CAPS = [4352, 4352, 4352, 512]  # tn jobs: 12288 total
W_CAP_SUM = sum(W_CAPS)  # 18304
WP_CAP_SUM = sum(WP_CAPS)  # 13568
DOTS_COLS = WP_CAP_SUM // P
NCH = 4  # re-gather chunks (4 m-values each)
CH_JOBS = B_LOC * CTX // NCH  # 4096

f32 = mybir.dt.float32
i32 = mybir.dt.int32
i16 = mybir.dt.int16

_cache = {}


def _build():
    nc = bacc.Bacc("TRN2", target_bir_lowering=False)

    D = nc.declare_dram_parameter("D", [N_DOCS, EMB], f32, isOutput=False)
    W = nc.declare_dram_parameter("W", [N_WORDS + 1, EMB], f32, isOutput=False)
    WpT = nc.declare_dram_parameter("WpT", [N_WORDS, EMB], f32, isOutput=False)
    docidx = nc.declare_dram_parameter("docidx", [P, M], i32, isOutput=False)
    wg_idx = nc.declare_dram_parameter(
        "wg_idx", [P, W_CAP_SUM // 16], i16, isOutput=False
    )
    rg_idx = nc.declare_dram_parameter(
        "rg_idx", [P, B_LOC * CTX // 16], i16, isOutput=False
    )
    yg_idx = nc.declare_dram_parameter(
        "yg_idx", [P, WP_CAP_SUM // 16], i16, isOutput=False
    )
    xg_idx = nc.declare_dram_parameter(
        "xg_idx", [P, WP_CAP_SUM // 16], i16, isOutput=False
    )
    counts = nc.declare_dram_parameter("counts", [1, 8], i32, isOutput=False)
    dots = nc.declare_dram_parameter("dots", [P, DOTS_COLS], f32, isOutput=True)
    xdump = nc.declare_dram_parameter("xdump", [P, M * EMB], f32, isOutput=True)

    x_hbm = nc.dram_tensor("x_hbm", [P, M, EMB], f32)
    w_scr = nc.dram_tensor("w_scr", [W_CAP_SUM, EMB], f32)

    w_off = [0]
    for c in W_CAPS:
        w_off.append(w_off[-1] + c)
    wp_off = [0]
    for c in WP_CAPS:
        wp_off.append(wp_off[-1] + c)

    stg_cap = max(W_CAPS)

    with (
        nc.Block() as block,
        ExitStack() as st,
    ):
        stg = [
            st.enter_context(
                nc.sbuf_tensor(f"stg{i}", [P, (stg_cap // P) * EMB], f32)
            )
            for i in range(2)
        ]
        WR = [
            st.enter_context(
                nc.sbuf_tensor(f"WR{i}", [P, (CH_JOBS // P) * EMB], f32)
            )
            for i in range(2)
        ]
        ycap = max(WP_CAPS)
        Y = [
            st.enter_context(nc.sbuf_tensor(f"Y{i}", [P, (ycap // P) * EMB], f32))
            for i in range(2)
        ]
        X = [
            st.enter_context(nc.sbuf_tensor(f"X{i}", [P, (ycap // P) * EMB], f32))
            for i in range(2)
        ]
        xD = st.enter_context(nc.sbuf_tensor("xD", [P, M * EMB], f32))
        xF = st.enter_context(nc.sbuf_tensor("xF", [P, M * EMB], f32))
        doc_t = st.enter_context(nc.sbuf_tensor("doc_t", [P, M], i32))
        wg_t = st.enter_context(nc.sbuf_tensor("wg_t", [P, W_CAP_SUM // 16], i16))
        rg_t = st.enter_context(
            nc.sbuf_tensor("rg_t", [P, B_LOC * CTX // 16], i16)
        )
        yg_t = st.enter_context(nc.sbuf_tensor("yg_t", [P, WP_CAP_SUM // 16], i16))
        xg_t = st.enter_context(nc.sbuf_tensor("xg_t", [P, WP_CAP_SUM // 16], i16))
        cnt_t = st.enter_context(nc.sbuf_tensor("cnt_t", [1, 8], i32))
        dots_t = st.enter_context(nc.sbuf_tensor("dots_t", [P, DOTS_COLS], f32))

        io = st.enter_context(nc.semaphore("io"))
        d_sem = st.enter_context(nc.semaphore("d_sem"))
        wg_sem = [
            st.enter_context(nc.semaphore(f"wg_sem{b}")) for b in range(W_BANKS)
        ]
        wsc_sem = [
            st.enter_context(nc.semaphore(f"wsc_sem{b}")) for b in range(W_BANKS)
        ]
        rg_sem = [
            st.enter_context(nc.semaphore(f"rg_sem{c}")) for c in range(NCH)
        ]
        xr_sem = st.enter_context(nc.semaphore("xr_sem"))
        xw_sem = st.enter_context(nc.semaphore("xw_sem"))
        yg_sem = [
            st.enter_context(nc.semaphore(f"yg_sem{b}")) for b in range(WP_BANKS)
        ]
        xg_sem = [
            st.enter_context(nc.semaphore(f"xg_sem{b}")) for b in range(WP_BANKS)
        ]
        pm_sem = st.enter_context(nc.semaphore("pm_sem"))
        v_sem = st.enter_context(nc.semaphore("v_sem"))
        fin_sem = st.enter_context(nc.semaphore("fin_sem"))

        @block.sync
        def _(sync: bass.BassEngine):
            sync.dma_start(doc_t[:], docidx[:]).then_inc(io, 16)
            sync.dma_start(wg_t[:], wg_idx[:]).then_inc(io, 16)
            sync.dma_start(rg_t[:], rg_idx[:]).then_inc(io, 16)
            sync.dma_start(yg_t[:], yg_idx[:]).then_inc(io, 16)
            sync.dma_start(xg_t[:], xg_idx[:]).then_inc(io, 16)
            sync.dma_start(cnt_t[:], counts[:]).then_inc(io, 16)
            # staging -> w_scr contiguous writes, per bank
            for b in range(W_BANKS):
                sync.wait_ge(wg_sem[b], 16)
                sync.dma_start(
                    w_scr[w_off[b] : w_off[b + 1], :],
                    stg[b % 2][:, : (W_CAPS[b] // P) * EMB].rearrange(
                        "p (r e) -> p r e", r=W_CAPS[b] // P, e=EMB
                    ),
                ).then_inc(wsc_sem[b], 16)
            # x -> x_hbm once the 4 reduce chunks are done
            sync.wait_ge(xr_sem, NCH)
            sync.dma_start(
                x_hbm[:].rearrange("p m e -> p (m e)"), xF[:]
            ).then_inc(xw_sem, 16)
            sync.dma_start(xdump[:], xF[:]).then_inc(fin_sem, 16)
            sync.wait_ge(v_sem, WP_BANKS)
            sync.dma_start(dots[:], dots_t[:]).then_inc(fin_sem, 16)
            sync.wait_ge(fin_sem, 32)

        @block.gpsimd
        def _(gpsimd: bass.BassGpSimd):
            gpsimd.load_library(mlp)
            gpsimd.wait_ge(io, 96)
            # --- stage A: D rows into batch-aligned xD ---
            for m in range(M):
                gpsimd.indirect_dma_start(
                    out=xD[:, m * EMB : (m + 1) * EMB],
                    out_offset=None,
                    in_=D[:],
                    in_offset=bass.IndirectOffsetOnAxis(
                        ap=doc_t[:, m : m + 1], axis=0
                    ),
                ).then_inc(d_sem, 16)
            with gpsimd.register("cnt") as cnt:
                # --- stage A: W bank gathers into rotating staging ---
                for b in range(W_BANKS):
                    gpsimd.reg_load(cnt, cnt_t[0:1, b : b + 1])
                    if b >= 2:
                        gpsimd.wait_ge(wsc_sem[b - 2], 16)
                    hi = min(BANK * (b + 1), N_WORDS + 1)
                    gpsimd.dma_gather(
                        stg[b % 2][:, : (W_CAPS[b] // P) * EMB].rearrange(
                            "p (r e) -> p r e", r=W_CAPS[b] // P, e=EMB
                        ),
                        W[BANK * b : hi, :],
                        wg_t[:, w_off[b] // 16 : w_off[b + 1] // 16],
                        W_CAPS[b],
                        cnt,
                        EMB,
                        single_packet=False,
                    ).then_inc(wg_sem[b], 16)
                # --- stage B: Y gathers for banks 0,1 (independent) ---
                for b in range(2):
                    gpsimd.reg_load(cnt, cnt_t[0:1, 4 + b : 5 + b])
                    hi = min(BANK * (b + 1), N_WORDS)
                    gpsimd.dma_gather(
                        Y[b % 2][:, : (WP_CAPS[b] // P) * EMB].rearrange(
                            "p (r e) -> p r e", r=WP_CAPS[b] // P, e=EMB
                        ),
                        WpT[BANK * b : hi, :],
                        yg_t[:, wp_off[b] // 16 : wp_off[b + 1] // 16],
                        WP_CAPS[b],
                        cnt,
                        EMB,
                        single_packet=False,
                    ).then_inc(yg_sem[b], 16)
                # --- stage A: re-gather from w_scr in (b,c) aligned order ---
                for b in range(W_BANKS):
                    gpsimd.wait_ge(wsc_sem[b], 16)
                for ch in range(NCH):
                    if ch >= 2:
                        gpsimd.wait_ge(xr_sem, ch - 1)
                    gpsimd.dma_gather(
                        WR[ch % 2][:].rearrange(
                            "p (r e) -> p r e", r=CH_JOBS // P, e=EMB
                        ),
                        w_scr[:],
                        rg_t[
                            :,
                            ch * (CH_JOBS // 16) : (ch + 1) * (CH_JOBS // 16),
                        ],
                        CH_JOBS,
                        CH_JOBS,
                        EMB,
                        single_packet=False,
                    ).then_inc(rg_sem[ch], 16)
                # --- stage B: X gathers + remaining Y gathers ---
                for b in range(WP_BANKS):
                    if b >= 2:
                        gpsimd.reg_load(cnt, cnt_t[0:1, 4 + b : 5 + b])
                        gpsimd.wait_ge(v_sem, b - 1)
                        hi = min(BANK * (b + 1), N_WORDS)
                        gpsimd.dma_gather(
                            Y[b % 2][:, : (WP_CAPS[b] // P) * EMB].rearrange(
                                "p (r e) -> p r e", r=WP_CAPS[b] // P, e=EMB
                            ),
                            WpT[BANK * b : hi, :],
                            yg_t[:, wp_off[b] // 16 : wp_off[b + 1] // 16],
                            WP_CAPS[b],
                            cnt,
                            EMB,
                            single_packet=False,
                        ).then_inc(yg_sem[b], 16)
                    gpsimd.reg_load(cnt, cnt_t[0:1, 4 + b : 5 + b])
                    gpsimd.wait_ge(xw_sem, 16)
                    if b >= 2:
                        gpsimd.wait_ge(v_sem, b - 1)
                    gpsimd.dma_gather(
                        X[b % 2][:, : (WP_CAPS[b] // P) * EMB].rearrange(
                            "p (r e) -> p r e", r=WP_CAPS[b] // P, e=EMB
                        ),
                        x_hbm[:].rearrange("p m e -> (p m) e"),
                        xg_t[:, wp_off[b] // 16 : wp_off[b + 1] // 16],
                        WP_CAPS[b],
                        cnt,
                        EMB,
                        single_packet=False,
                    ).then_inc(xg_sem[b], 16)

        @block.vector
        def _(vector: bass.BassEngine):
            # stage A: reduce re-gathered ctx rows (8 per b) + add D rows
            vector.wait_ge(d_sem, 16 * M)
            for ch in range(NCH):
                vector.wait_ge(rg_sem[ch], 16)
                mlo = ch * (M // NCH)
                # WR chunk layout: [p, (m4, c8), e]; reduce over c
                src = WR[ch % 2][:].rearrange(
                    "p (m c e) -> p m e c", m=M // NCH, c=CTX, e=EMB
                )
                xslice = xF[:, mlo * EMB : (mlo + M // NCH) * EMB]
                vector.tensor_reduce(
                    out=xslice.rearrange("p (m e) -> p m e", m=M // NCH, e=EMB),
                    in_=src,
                    axis=mybir.AxisListType.X,
                    op=mybir.AluOpType.add,
                ).then_inc(pm_sem, 1)
                vector.wait_ge(pm_sem, ch + 1)
                vector.tensor_tensor(
                    out=xslice,
                    in0=xslice,
                    in1=xD[:, mlo * EMB : (mlo + M // NCH) * EMB],
                    op=mybir.AluOpType.add,
                ).then_inc(xr_sem, 1)
            # stage B: dots
            for b in range(WP_BANKS):
                vector.wait_ge(yg_sem[b], 16)
                vector.wait_ge(xg_sem[b], 16)
                n = WP_CAPS[b] // P
                Xv = X[b % 2][:, : n * EMB]
                Yv = Y[b % 2][:, : n * EMB]
                vector.tensor_tensor(
                    out=Xv, in0=Xv, in1=Yv, op=mybir.AluOpType.mult
                ).then_inc(pm_sem, 1)
                vector.wait_ge(pm_sem, NCH + b + 1)
                vector.tensor_reduce(
                    out=dots_t[:, wp_off[b] // P : wp_off[b + 1] // P],
                    in_=X[b % 2][:].rearrange(
                        "p (r e) -> p r e", r=ycap // P, e=EMB
                    )[:, :n, :],
                    axis=mybir.AxisListType.X,
                    op=mybir.AluOpType.add,
                ).then_inc(v_sem, 1)

    nc.compile()
    return nc


def _wrap(lst, cap):
    """int16 job list -> [128, cap//16] wrapped (i at [i%16, i//16]) and
    replicated across the 8 gpsimd cores."""
    padded = np.full(cap, -1, dtype=np.int16)
    padded[: len(lst)] = lst
    w = padded.reshape(cap // 16, 16).T  # [16, cap//16]
    return np.tile(w, (8, 1))


LAST_RESULTS = None


def kernel(D, W, Wp, ctx_ids, doc_ids, target_and_noise_ids):
    global LAST_RESULTS
    if "nc" not in _cache:
        _cache["nc"] = _build()
    nc = _cache["nc"]

    D = np.ascontiguousarray(np.asarray(D, dtype=np.float32))
    W = np.ascontiguousarray(np.asarray(W, dtype=np.float32))
    WpT = np.ascontiguousarray(np.asarray(Wp, dtype=np.float32).T)
    ctx64 = np.asarray(ctx_ids, dtype=np.int64)
    doc64 = np.asarray(doc_ids, dtype=np.int64)
    tn64 = np.asarray(target_and_noise_ids, dtype=np.int64)

    w_off_np = np.concatenate([[0], np.cumsum(W_CAPS)])
    wp_off_np = np.concatenate([[0], np.cumsum(WP_CAPS)])

    in_maps = []
    perms = []
    for c in range(N_CORES):
        sl = slice(c * B_LOC, (c + 1) * B_LOC)
        doc_l = doc64[sl]
        ctx_l = ctx64[sl]
        tn_l = tn64[sl]

        docidx = doc_l.reshape(M, P).T.astype(np.int32)

        # W ctx jobs sorted by bank; job index j = b*8 + c
        ids = ctx_l.ravel()
        bank = (ids >> 15).astype(np.int64)
        order = np.argsort(bank, kind="stable")
        wcounts = np.bincount(bank, minlength=W_BANKS)
        assert (wcounts <= np.array(W_CAPS)).all(), wcounts
        g16 = (ids[order] & 32767).astype(np.int16)
        wg_parts, pos = [], 0
        for b in range(W_BANKS):
            n = wcounts[b]
            wg_parts.append(_wrap(g16[pos : pos + n], W_CAPS[b]))
            pos += n
        wg_idx = np.concatenate(wg_parts, axis=1)

        # scratch slot of job j (in original j = b*8+c order).  The
        # staging->scratch DMA linearizes partition-major, so gather list
        # position i lands at scratch row (i%128)*(cap/128) + i//128.
        slot = np.empty(B_LOC * CTX, dtype=np.int64)
        pos = 0
        for b in range(W_BANKS):
            n = wcounts[b]
            i = np.arange(n)
            rows = (i % P) * (W_CAPS[b] // P) + i // P
            slot[order[pos : pos + n]] = w_off_np[b] + rows
            pos += n
        # re-gather position j' = s*128 + p, p = b%128, s = (b//128)*8 + c
        jj = np.arange(B_LOC * CTX)
        bb, cc = jj // CTX, jj % CTX
        jprime = ((bb // P) * CTX + cc) * P + (bb % P)
        rg = np.empty(B_LOC * CTX, dtype=np.int16)
        rg[jprime] = slot[jj].astype(np.int16)
        rg_idx = _wrap(rg, B_LOC * CTX)

        # Wp jobs sorted by bank
        ids2 = tn_l.ravel()
        bank2 = (ids2 >> 15).astype(np.int64)
        order2 = np.argsort(bank2, kind="stable")
        ycounts = np.bincount(bank2, minlength=WP_BANKS)
        assert (ycounts <= np.array(WP_CAPS)).all(), ycounts
        yg16 = (ids2[order2] & 32767).astype(np.int16)
        bdest = order2 // K
        xg16 = ((bdest & 127) * M + (bdest >> 7)).astype(np.int16)
        yg_parts, xg_parts, pos = [], [], 0
        for b in range(WP_BANKS):
            n = ycounts[b]
            yg_parts.append(_wrap(yg16[pos : pos + n], WP_CAPS[b]))
            xg_parts.append(_wrap(xg16[pos : pos + n], WP_CAPS[b]))
            pos += n
        yg_idx = np.concatenate(yg_parts, axis=1)
        xg_idx = np.concatenate(xg_parts, axis=1)

        cnt = np.zeros((1, 8), dtype=np.int32)
        cnt[0, :4] = wcounts
        cnt[0, 4:8] = ycounts

        perms.append((order2, ycounts))
        in_maps.append(
            {
                "D": D,
                "W": W,
                "WpT": WpT,
                "docidx": docidx,
                "wg_idx": wg_idx,
                "rg_idx": rg_idx,
                "yg_idx": yg_idx,
                "xg_idx": xg_idx,
                "counts": cnt,
            }
        )

    res = run_bass_kernel_spmd(nc, in_maps, list(range(N_CORES)))
    LAST_RESULTS = res

    out = np.empty((B, K), dtype=np.float32)
    for c in range(N_CORES):
        dots = res.results[c]["dots"]  # [128, DOTS_COLS]
        order2, ycounts = perms[c]
        vals = np.empty(B_LOC * K, dtype=np.float32)
        pos = 0
        for b in range(WP_BANKS):
            n = ycounts[b]
            j = np.arange(n)
            cols = wp_off_np[b] // P + j // P
            vals[order2[pos : pos + n]] = dots[j % P, cols]
            pos += n
        out[c * B_LOC : (c + 1) * B_LOC] = vals.reshape(B_LOC, K)
    return out
